# revision 32
# baseline (speedup 1.0000x reference)
"""Trainium2 Bass kernel for nn_DualAgentEnvironment (dual encoder/decoder CE + MI reward).

Data-parallel over B=4096 -> 8 cores x 512 rows. Each core runs a
feature-major MLP pipeline (features on partitions, tokens on the free dim),
so every layer is one PE matmul with no transposes. Host-side folds:
  - LN gain into the decoder weight; LN mean removal by column-centering it
    (the rank-1 mean correction ones (x) u equals subtracting column means)
  - LN beta + first decoder bias into one per-feature bias
  - logits head into a difference head dw = w2[:,1]-w2[:,0] (2-class CE only
    needs the logit gap: ce = softplus((l1-l0)*(1-2y)))
Device LN reduces to a per-token inv-std column scale applied after the
decoder matmul (column scaling commutes with left matmul). Per-token LN
sums are accumulated into an aligned [4,512] PSUM tile via selector-column
matmuls, batched per 8-tile group in a PSUM slab (PSUM access patterns are
exempt from the SBUF 32-partition-alignment rule, enabling shifted-row
reads), and turned into inv-std with one batched sqrt + per-tile reciprocal.
The device outputs per-token logit gaps [64,2,512]; the cheap O(B*T)
softplus/mask/entropy/reward/aux math runs on the host.

kernel(**inputs) takes FULL unsharded inputs, returns
(reward [4096,64] f32, aux_loss f32 scalar), matching reference().
"""

import numpy as np
import ml_dtypes
from contextlib import ExitStack

import concourse.bass as bass
import concourse.bacc as bacc
import concourse.mybir as mybir
import concourse.tile as tile
from concourse.bass_utils import run_bass_kernel_spmd

F32 = mybir.dt.float32
F32R = mybir.dt.float32r
BF16 = mybir.dt.bfloat16
AX = mybir.AxisListType
OP = mybir.AluOpType
ACTF = mybir.ActivationFunctionType

B, T, F, H = 4096, 64, 128, 128
LAM = 0.5
NCORES = 8
BS = B // NCORES            # 512 batch rows per core
TOK = BS * T                # 32768 tokens per core
NTOK = 512                  # tokens per tile
NT = TOK // NTOK            # 64 tiles
GT = 8                      # tiles per group (stats batching)
NG = NT // GT               # 8 groups

# weights pack column offsets (f32 [128, WC])
_C_CEW2 = 0
_C_SEW2 = 128
_C_WGCD = 256
_C_JDT = 384
_C_JDB = 512
_C_DWC = 640          # [128,2]: col0 = cd_w2[:,1]-cd_w2[:,0], col1 = 0
_C_DWJ = 642          # [128,2]: col0 = 0, col1 = jd_w2[:,1]-jd_w2[:,0]
_C_SEL = 644          # 4 selector blocks [128,2] for the LN stat sums
WC = 652

# bias pack columns (f32 [128, NBP])
(_B_CE1, _B_CE2, _B_SE1, _B_SE2, _B_CD1, _B_JD1,
 _B_KVG, _B_KVJ, _B_EPS) = range(9)
NBP = 10


def _f32r(ap):
    return ap.bitcast(F32R)


def _build_kernel(nc: bass.Bass):
    xc = nc.dram_tensor("xc", [TOK, F], BF16, kind="ExternalInput").ap()
    xs = nc.dram_tensor("xs", [TOK, F], BF16, kind="ExternalInput").ap()
    wts = nc.dram_tensor("wts", [128, WC], F32R, kind="ExternalInput").ap()
    wbf = nc.dram_tensor("wbf", [128, 256], BF16, kind="ExternalInput").ap()
    biasp = nc.dram_tensor("biasp", [128, NBP], F32, kind="ExternalInput").ap()
    d_out = nc.dram_tensor("d", [NG, 2, GT * NTOK], F32, kind="ExternalOutput").ap()
    ustg = nc.dram_tensor("ustg", [NG, 2, GT * NTOK], F32).ap()
    sstg = nc.dram_tensor("sstg", [NG, 2 * GT, NTOK], BF16).ap()

    with tile.TileContext(nc) as tc, ExitStack() as ctx:
        consts = ctx.enter_context(tc.tile_pool(name="consts", bufs=1))
        xin = ctx.enter_context(tc.tile_pool(name="xin", bufs=2))
        sb1 = ctx.enter_context(tc.tile_pool(name="sb1", bufs=2))
        rep = ctx.enter_context(tc.tile_pool(name="rep", bufs=16))
        bcp = ctx.enter_context(tc.tile_pool(name="bcp", bufs=3))
        statp = ctx.enter_context(tc.tile_pool(name="statp", bufs=2))
        dsm = ctx.enter_context(tc.tile_pool(name="dsm", bufs=2))
        pmm = ctx.enter_context(tc.tile_pool(name="pmm", bufs=3, space="PSUM"))
        psml = ctx.enter_context(tc.tile_pool(name="psml", bufs=2, space="PSUM"))
        pz2 = ctx.enter_context(tc.tile_pool(name="pz2", bufs=2, space="PSUM"))
        plg2 = ctx.enter_context(tc.tile_pool(name="plg2", bufs=1, space="PSUM"))

        # constants
        W = consts.tile([128, WC], F32R, tag="wts")
        nc.sync.dma_start(W[:], wts)
        WB = consts.tile([128, 256], BF16, tag="wbf")
        nc.sync.dma_start(WB[:], wbf)
        BP = consts.tile([128, NBP], F32, tag="biasp")
        nc.sync.dma_start(BP[:], biasp)

        sel = lambda k: W[:, _C_SEL + 2 * k:_C_SEL + 2 * (k + 1)]

        for g in range(NG):
            creps, sreps = [], []
            Uslab = dsm.tile([2, GT * NTOK], F32, tag="Uslab")
            # ---- phase A: encoders + LN stat sums, super-tiles of 4 ----
            for st in range(GT // 2):
                base = g * GT + st * 2
                xc4 = xin.tile([128, 2 * NTOK], BF16, tag="xc4")
                nc.sync.dma_start_transpose(xc4[:], xc[base * NTOK:(base + 2) * NTOK, :])
                xs4 = xin.tile([128, 2 * NTOK], BF16, tag="xs4")
                nc.sync.dma_start_transpose(xs4[:], xs[base * NTOK:(base + 2) * NTOK, :])
                for j in range(2):
                    xcT = xc4[:, j * NTOK:(j + 1) * NTOK]
                    xsT = xs4[:, j * NTOK:(j + 1) * NTOK]
                    c1 = pmm.tile([128, NTOK], F32, tag="mm")
                    nc.tensor.matmul(c1[:], WB[:, 0:128], xcT, start=True, stop=True)
                    g1c = sb1.tile([128, NTOK], F32R, tag="g1c")
                    nc.scalar.activation(g1c[:], c1[:], ACTF.Gelu, bias=BP[:, _B_CE1:_B_CE1 + 1])
                    c2 = pmm.tile([128, NTOK], F32, tag="mm")
                    nc.tensor.matmul(c2[:], W[:, _C_CEW2:_C_CEW2 + 128], g1c[:], start=True, stop=True)
                    crep = rep.tile([128, NTOK], F32R, tag="crep")
                    nc.vector.tensor_scalar(crep[:], c2[:], BP[:, _B_CE2:_B_CE2 + 1], None, op0=OP.add)
                    sqc = sb1.tile([128, NTOK], F32R, tag="sqc")
                    nc.gpsimd.tensor_tensor(sqc[:], crep[:], crep[:], op=OP.mult)

                    s1 = pmm.tile([128, NTOK], F32, tag="mm")
                    nc.tensor.matmul(s1[:], WB[:, 128:256], xsT, start=True, stop=True)
                    g1s = sb1.tile([128, NTOK], F32R, tag="g1s")
                    nc.scalar.activation(g1s[:], s1[:], ACTF.Gelu, bias=BP[:, _B_SE1:_B_SE1 + 1])
                    s2 = pmm.tile([128, NTOK], F32, tag="mm")
                    nc.tensor.matmul(s2[:], W[:, _C_SEW2:_C_SEW2 + 128], g1s[:], start=True, stop=True)
                    srep = rep.tile([128, NTOK], F32R, tag="srep")
                    nc.vector.tensor_scalar(srep[:], s2[:], BP[:, _B_SE2:_B_SE2 + 1], None, op0=OP.add)
                    sqs = sb1.tile([128, NTOK], F32R, tag="sqs")
                    nc.gpsimd.tensor_tensor(sqs[:], srep[:], srep[:], op=OP.mult)

                    # LN stats: SPA = (sum_c, sum_j), SPB = (H ssq_c, 2H ssq_j)
                    SPA = psml.tile([2, NTOK], F32, tag="sml")
                    nc.tensor.matmul(SPA[:], sel(0), crep[:], start=True, stop=False)
                    nc.tensor.matmul(SPA[:], sel(1), srep[:], start=False, stop=True)
                    SPB = psml.tile([2, NTOK], F32, tag="sml")
                    nc.tensor.matmul(SPB[:], sel(2), sqc[:], start=True, stop=False)
                    nc.tensor.matmul(SPB[:], sel(3), sqs[:], start=False, stop=True)
                    # u = n*ssq - sum^2, packed into the group slab (free dim)
                    T1 = sb1.tile([2, NTOK], F32, tag="T1")
                    nc.scalar.activation(T1[:], SPA[:], ACTF.Square)
                    i = st * 2 + j
                    nc.vector.tensor_tensor(Uslab[:, i * NTOK:(i + 1) * NTOK], SPB[:], T1[:], op=OP.subtract)
                    creps.append(crep)
                    sreps.append(srep)

            # batched inv-std for the group via the DRAM staging round-trip
            nc.sync.dma_start(ustg[g], Uslab[:])
            USL = statp.tile([2 * GT, NTOK], F32, tag="USL")
            nc.sync.dma_start(USL[:], ustg[g].rearrange("r (i c) -> i r c", c=NTOK))
            SD = statp.tile([2 * GT, NTOK], F32, tag="SD")
            nc.scalar.activation(SD[:], USL[:], ACTF.Sqrt, bias=BP[0:2 * GT, _B_EPS:_B_EPS + 1])
            S16 = statp.tile([2 * GT, NTOK], BF16, tag="S16")
            with nc.allow_low_precision(reason="inv-std broadcast in bf16"):
                nc.vector.reciprocal(S16[:], SD[:])
            nc.sync.dma_start(sstg[g], S16[:])

            # ---- phase C: decoders, super-tiles of 4 ----
            Dslab = dsm.tile([2, GT * NTOK], F32, tag="Dslab")
            for i in range(GT):
                if True:
                    n = g * GT + i
                    crep, srep = creps[i], sreps[i]
                    SBC = bcp.tile([128, 2, NTOK], BF16, tag="sbc")
                    nc.sync.dma_start(SBC[:], sstg[g, 2 * i:2 * i + 2, :].unsqueeze(0).to_broadcast((128, 2, NTOK)))

                    Zcd = pz2.tile([128, NTOK], F32, tag="Z")
                    nc.tensor.matmul(Zcd[:], W[:, _C_WGCD:_C_WGCD + 128], crep[:], start=True, stop=True)
                    zc = sb1.tile([128, NTOK], F32, tag="zc")
                    nc.vector.tensor_tensor(zc[:], Zcd[:], SBC[:, 0, :], op=OP.mult)
                    gcd = sb1.tile([128, NTOK], F32R, tag="gcd")
                    nc.scalar.activation(gcd[:], zc[:], ACTF.Gelu, bias=BP[:, _B_CD1:_B_CD1 + 1])

                    Zjd = pz2.tile([128, NTOK], F32, tag="Z")
                    nc.tensor.matmul(Zjd[:], W[:, _C_JDT:_C_JDT + 128], crep[:], start=True, stop=False)
                    nc.tensor.matmul(Zjd[:], W[:, _C_JDB:_C_JDB + 128], srep[:], start=False, stop=True)
                    zj = sb1.tile([128, NTOK], F32, tag="zj")
                    nc.vector.tensor_tensor(zj[:], Zjd[:], SBC[:, 1, :], op=OP.mult)
                    gjd = sb1.tile([128, NTOK], F32R, tag="gjd")
                    nc.scalar.activation(gjd[:], zj[:], ACTF.Gelu, bias=BP[:, _B_JD1:_B_JD1 + 1])

                    LG = plg2.tile([2, NTOK], F32, tag="lg")
                    nc.tensor.matmul(LG[:], W[:, _C_DWC:_C_DWC + 2], gcd[:], start=True, stop=False)
                    nc.tensor.matmul(LG[:], W[:, _C_DWJ:_C_DWJ + 2], gjd[:], start=False, stop=True)
                    if i % 2 == 0:
                        nc.vector.tensor_copy(Dslab[:, i * NTOK:(i + 1) * NTOK], LG[:])
                    else:
                        nc.scalar.copy(Dslab[:, i * NTOK:(i + 1) * NTOK], LG[:])
            nc.sync.dma_start(d_out[g], Dslab[:])
    return nc


_CACHED = {}


def _get_nc():
    if "nc" not in _CACHED:
        nc = bacc.Bacc("TRN2", target_bir_lowering=False, debug=False)
        _build_kernel(nc)
        nc.compile()
        _CACHED["nc"] = nc
    return _CACHED["nc"]


def _host_prep(inputs):
    f32 = np.float32
    g = lambda k: np.asarray(inputs[k], f32)
    xc = g('causal_summary').reshape(B * T, F)
    xs = g('spurious_summary').reshape(B * T, F)
    labels = np.asarray(inputs['labels']).astype(np.int64)
    mask = np.asarray(inputs['valid_mask']).astype(f32)

    onehot = np.eye(2, dtype=f32)[labels]
    counts = (mask.T @ onehot).astype(f32)
    tot = counts.sum(-1)
    probs = counts / np.clip(tot, 1.0, None)[:, None]
    ent = -(probs * np.log(probs + 1e-8)).sum(-1).astype(f32)
    sv = (tot >= 2.0).astype(f32)
    lab2 = (1.0 - 2.0 * labels).astype(f32)

    cd_g, cd_b = g('cd_g'), g('cd_b')
    jd_g, jd_b = g('jd_g'), g('jd_b')
    cd_w1, jd_w1 = g('cd_w1'), g('jd_w1')
    # fold LN gain, then LN mean removal = column-centering
    Wg_cd = (cd_w1 * cd_g[:, None]).astype(f32)
    Wg_cd = (Wg_cd - Wg_cd.mean(0, keepdims=True)) * H
    bias_cd1 = (g('cd_b1') + cd_w1.T @ cd_b).astype(f32)
    Wg_jd = (jd_w1 * jd_g[:, None]).astype(f32)
    Wg_jd = (Wg_jd - Wg_jd.mean(0, keepdims=True)) * (2 * H)
    bias_jd1 = (g('jd_b1') + jd_w1.T @ jd_b).astype(f32)
    cd_w2, jd_w2 = g('cd_w2'), g('jd_w2')
    cd_b2, jd_b2 = g('cd_b2'), g('jd_b2')
    dw_cd = (cd_w2[:, 1] - cd_w2[:, 0]).astype(f32)
    dw_jd = (jd_w2[:, 1] - jd_w2[:, 0]).astype(f32)
    db_cd = f32(cd_b2[1] - cd_b2[0])
    db_jd = f32(jd_b2[1] - jd_b2[0])

    wts = np.zeros((128, WC), f32)
    wts[:, _C_CEW2:_C_CEW2 + 128] = g('ce_w2')
    wts[:, _C_SEW2:_C_SEW2 + 128] = g('se_w2')
    wts[:, _C_WGCD:_C_WGCD + 128] = Wg_cd
    wts[:, _C_JDT:_C_JDT + 128] = Wg_jd[:H]
    wts[:, _C_JDB:_C_JDB + 128] = Wg_jd[H:]
    wts[:, _C_DWC] = dw_cd
    wts[:, _C_DWJ + 1] = dw_jd
    # stat selectors: SPA = (sum_c, sum_c+sum_s), SPB = (H ssq_c, 2H (ssq_c+ssq_s))
    wts[:, _C_SEL + 0] = 1.0
    wts[:, _C_SEL + 1] = 1.0
    wts[:, _C_SEL + 3] = 1.0
    wts[:, _C_SEL + 4] = H
    wts[:, _C_SEL + 5] = 2 * H
    wts[:, _C_SEL + 7] = 2 * H

    wbf = np.zeros((128, 256), ml_dtypes.bfloat16)
    wbf[:, 0:128] = g('ce_w1').astype(ml_dtypes.bfloat16)
    wbf[:, 128:256] = g('se_w1').astype(ml_dtypes.bfloat16)

    biasp = np.zeros((128, NBP), f32)
    biasp[:, _B_CE1] = g('ce_b1')
    biasp[:, _B_CE2] = g('ce_b2')
    biasp[:, _B_SE1] = g('se_b1')
    biasp[:, _B_SE2] = g('se_b2')
    biasp[:, _B_CD1] = bias_cd1
    biasp[:, _B_JD1] = bias_jd1
    biasp[0:2 * GT:2, _B_EPS] = (H * H) * 1e-5
    biasp[1:2 * GT:2, _B_EPS] = (2 * H) * (2 * H) * 1e-5

    bf = ml_dtypes.bfloat16
    in_maps = []
    for core in range(NCORES):
        s = core * TOK
        in_maps.append({
            "xc": np.ascontiguousarray(xc[s:s + TOK]).astype(bf),
            "xs": np.ascontiguousarray(xs[s:s + TOK]).astype(bf),
            "wts": wts, "wbf": wbf, "biasp": biasp,
        })
    host = {"ent": ent, "sv": sv, "tot": tot, "mask": mask,
            "lab2": lab2, "db_cd": db_cd, "db_jd": db_jd}
    return in_maps, host


def _softplus(x):
    return np.log1p(np.exp(-np.abs(x))) + np.maximum(x, 0)


def _host_post(results, host):
    f32 = np.float32
    ent, sv, tot = host["ent"], host["sv"], host["tot"]
    mask, lab2 = host["mask"], host["lab2"]
    d_c = np.concatenate([np.asarray(r["d"], f32)[:, 0, :].reshape(TOK) for r in results])
    d_j = np.concatenate([np.asarray(r["d"], f32)[:, 1, :].reshape(TOK) for r in results])
    lab2f = np.repeat(lab2, T)
    ce_c = _softplus((d_c + host["db_cd"]) * lab2f).reshape(B, T).astype(f32)
    ce_j = _softplus((d_j + host["db_jd"]) * lab2f).reshape(B, T).astype(f32)
    reward = mask * sv[None, :] * (ent[None, :] - ce_c - LAM * np.maximum(ce_c - ce_j, 0.0))
    Sc = (mask * ce_c).sum(0)
    Sj = (mask * ce_j).sum(0)
    denom = np.clip(tot, 1.0, None)
    aux_step = 0.5 * (Sc + Sj) / denom
    n_valid = sv.sum()
    aux = (sv * aux_step).sum() / max(n_valid, 1.0) if n_valid > 0 else 0.0
    return reward.astype(f32), np.float32(aux)


def kernel(**inputs):
    nc = _get_nc()
    in_maps, host = _host_prep(inputs)
    res = run_bass_kernel_spmd(nc, in_maps, list(range(NCORES)))
    return _host_post(res.results, host)


if __name__ == "__main__":
    import reference as ref
    inputs = {k: np.asarray(v) for k, v in ref.setup_inputs().items()}
    out = kernel(**inputs)
    print(out[0].shape, out[1])


# revision 34
# speedup vs baseline: 1.1564x; 1.1564x over previous
"""Trainium2 Bass kernel for nn_DualAgentEnvironment (dual encoder/decoder CE + MI reward).

Data-parallel over B=4096 -> 8 cores x 512 rows. Each core runs a
feature-major MLP pipeline (features on partitions, tokens on the free dim),
so every layer is one PE matmul with no transposes. Host-side folds:
  - LN gain into the decoder weight; LN mean removal by column-centering it
    (the rank-1 mean correction ones (x) u equals subtracting column means)
  - LN beta + first decoder bias into one per-feature bias
  - logits head into a difference head dw = w2[:,1]-w2[:,0] (2-class CE only
    needs the logit gap: ce = softplus((l1-l0)*(1-2y)))
Device LN reduces to a per-token inv-std column scale applied after the
decoder matmul (column scaling commutes with left matmul). Per-token LN
sums are accumulated into an aligned [4,512] PSUM tile via selector-column
matmuls, batched per 8-tile group in a PSUM slab (PSUM access patterns are
exempt from the SBUF 32-partition-alignment rule, enabling shifted-row
reads), and turned into inv-std with one batched sqrt + per-tile reciprocal.
The device outputs per-token logit gaps [64,2,512]; the cheap O(B*T)
softplus/mask/entropy/reward/aux math runs on the host.

kernel(**inputs) takes FULL unsharded inputs, returns
(reward [4096,64] f32, aux_loss f32 scalar), matching reference().
"""

import numpy as np
import ml_dtypes
from contextlib import ExitStack

import concourse.bass as bass
import concourse.bacc as bacc
import concourse.mybir as mybir
import concourse.tile as tile
from concourse.bass_utils import run_bass_kernel_spmd

F32 = mybir.dt.float32
F32R = mybir.dt.float32r
BF16 = mybir.dt.bfloat16
AX = mybir.AxisListType
OP = mybir.AluOpType
ACTF = mybir.ActivationFunctionType

B, T, F, H = 4096, 64, 128, 128
LAM = 0.5
NCORES = 8
BS = B // NCORES            # 512 batch rows per core
TOK = BS * T                # 32768 tokens per core
NTOK = 512                  # tokens per tile
NT = TOK // NTOK            # 64 tiles
GT = 8                      # tiles per group (stats batching)
NG = NT // GT               # 8 groups

# weights pack column offsets (f32 [128, WC])
_C_CEW2 = 0
_C_SEW2 = 128
_C_WGCD = 256
_C_JDT = 384
_C_JDB = 512
_C_DWC = 640          # [128,2]: col0 = cd_w2[:,1]-cd_w2[:,0], col1 = 0
_C_DWJ = 642          # [128,2]: col0 = 0, col1 = jd_w2[:,1]-jd_w2[:,0]
_C_SEL = 644          # 4 selector blocks [128,2] for the LN stat sums
WC = 652

# bias pack columns (f32 [128, NBP])
(_B_CE1, _B_CE2, _B_SE1, _B_SE2, _B_CD1, _B_JD1,
 _B_KVG, _B_KVJ, _B_EPS) = range(9)
NBP = 10


def _f32r(ap):
    return ap.bitcast(F32R)


def _build_kernel(nc: bass.Bass):
    xc = nc.dram_tensor("xc", [TOK, F], BF16, kind="ExternalInput").ap()
    xs = nc.dram_tensor("xs", [TOK, F], BF16, kind="ExternalInput").ap()
    wts = nc.dram_tensor("wts", [128, WC], F32R, kind="ExternalInput").ap()
    wbf = nc.dram_tensor("wbf", [128, 256], BF16, kind="ExternalInput").ap()
    biasp = nc.dram_tensor("biasp", [128, NBP], F32, kind="ExternalInput").ap()
    d_out = nc.dram_tensor("d", [NG, 2, GT * NTOK], F32, kind="ExternalOutput").ap()
    ustg = nc.dram_tensor("ustg", [NG, 2, GT * NTOK], F32).ap()
    sstg = nc.dram_tensor("sstg", [NG, 2 * GT, NTOK], BF16).ap()

    with tile.TileContext(nc) as tc, ExitStack() as ctx:
        consts = ctx.enter_context(tc.tile_pool(name="consts", bufs=1))
        xin = ctx.enter_context(tc.tile_pool(name="xin", bufs=2))
        sb1 = ctx.enter_context(tc.tile_pool(name="sb1", bufs=2))
        rep = ctx.enter_context(tc.tile_pool(name="rep", bufs=16))
        bcp = ctx.enter_context(tc.tile_pool(name="bcp", bufs=3))
        statp = ctx.enter_context(tc.tile_pool(name="statp", bufs=2))
        dsm = ctx.enter_context(tc.tile_pool(name="dsm", bufs=2))
        pmm = ctx.enter_context(tc.tile_pool(name="pmm", bufs=4, space="PSUM"))
        psml = ctx.enter_context(tc.tile_pool(name="psml", bufs=2, space="PSUM"))
        pz2 = ctx.enter_context(tc.tile_pool(name="pz2", bufs=1, space="PSUM"))
        plg2 = ctx.enter_context(tc.tile_pool(name="plg2", bufs=1, space="PSUM"))

        # constants
        W = consts.tile([128, WC], F32R, tag="wts")
        nc.sync.dma_start(W[:], wts)
        WB = consts.tile([128, 256], BF16, tag="wbf")
        nc.sync.dma_start(WB[:], wbf)
        BP = consts.tile([128, NBP], F32, tag="biasp")
        nc.sync.dma_start(BP[:], biasp)

        sel = lambda k: W[:, _C_SEL + 2 * k:_C_SEL + 2 * (k + 1)]

        for g in range(NG):
            creps, sreps = [], []
            Uslab = dsm.tile([2, GT * NTOK], F32, tag="Uslab")
            # ---- phase A: encoders + LN stat sums, super-tiles of 4 ----
            for st in range(GT // 2):
                base = g * GT + st * 2
                xc4 = xin.tile([128, 2 * NTOK], BF16, tag="xc4")
                nc.sync.dma_start_transpose(xc4[:], xc[base * NTOK:(base + 2) * NTOK, :])
                xs4 = xin.tile([128, 2 * NTOK], BF16, tag="xs4")
                nc.sync.dma_start_transpose(xs4[:], xs[base * NTOK:(base + 2) * NTOK, :])
                for j in range(2):
                    xcT = xc4[:, j * NTOK:(j + 1) * NTOK]
                    xsT = xs4[:, j * NTOK:(j + 1) * NTOK]
                    c1 = pmm.tile([128, NTOK], F32, tag="mm")
                    nc.tensor.matmul(c1[:], WB[:, 0:128], xcT, start=True, stop=True)
                    g1c = sb1.tile([128, NTOK], F32R, tag="g1c")
                    nc.scalar.activation(g1c[:], c1[:], ACTF.Gelu, bias=BP[:, _B_CE1:_B_CE1 + 1])
                    c2 = pmm.tile([128, NTOK], F32, tag="mm")
                    nc.tensor.matmul(c2[:], W[:, _C_CEW2:_C_CEW2 + 128], g1c[:], start=True, stop=True)
                    crep = rep.tile([128, NTOK], F32R, tag="crep")
                    nc.vector.tensor_scalar(crep[:], c2[:], BP[:, _B_CE2:_B_CE2 + 1], None, op0=OP.add)
                    sqc = sb1.tile([128, NTOK], F32R, tag="sqc")
                    nc.gpsimd.tensor_tensor(sqc[:], crep[:], crep[:], op=OP.mult)

                    s1 = pmm.tile([128, NTOK], F32, tag="mm")
                    nc.tensor.matmul(s1[:], WB[:, 128:256], xsT, start=True, stop=True)
                    g1s = sb1.tile([128, NTOK], F32R, tag="g1s")
                    nc.scalar.activation(g1s[:], s1[:], ACTF.Gelu, bias=BP[:, _B_SE1:_B_SE1 + 1])
                    s2 = pmm.tile([128, NTOK], F32, tag="mm")
                    nc.tensor.matmul(s2[:], W[:, _C_SEW2:_C_SEW2 + 128], g1s[:], start=True, stop=True)
                    srep = rep.tile([128, NTOK], F32R, tag="srep")
                    nc.vector.tensor_scalar(srep[:], s2[:], BP[:, _B_SE2:_B_SE2 + 1], None, op0=OP.add)
                    sqs = sb1.tile([128, NTOK], F32R, tag="sqs")
                    nc.gpsimd.tensor_tensor(sqs[:], srep[:], srep[:], op=OP.mult)

                    # LN stats: SPA = (sum_c, sum_j), SPB = (H ssq_c, 2H ssq_j)
                    SPA = psml.tile([2, NTOK], F32, tag="sml")
                    nc.tensor.matmul(SPA[:], sel(0), crep[:], start=True, stop=False)
                    nc.tensor.matmul(SPA[:], sel(1), srep[:], start=False, stop=True)
                    SPB = psml.tile([2, NTOK], F32, tag="sml")
                    nc.tensor.matmul(SPB[:], sel(2), sqc[:], start=True, stop=False)
                    nc.tensor.matmul(SPB[:], sel(3), sqs[:], start=False, stop=True)
                    # u = n*ssq - sum^2, packed into the group slab (free dim)
                    T1 = sb1.tile([2, NTOK], F32, tag="T1")
                    nc.scalar.activation(T1[:], SPA[:], ACTF.Square)
                    i = st * 2 + j
                    nc.vector.tensor_tensor(Uslab[:, i * NTOK:(i + 1) * NTOK], SPB[:], T1[:], op=OP.subtract)
                    creps.append(crep)
                    sreps.append(srep)

            # batched inv-std for the group via the DRAM staging round-trip
            nc.sync.dma_start(ustg[g], Uslab[:])
            USL = statp.tile([2 * GT, NTOK], F32, tag="USL")
            nc.sync.dma_start(USL[:], ustg[g].rearrange("r (i c) -> i r c", c=NTOK))
            SD = statp.tile([2 * GT, NTOK], F32, tag="SD")
            nc.scalar.activation(SD[:], USL[:], ACTF.Sqrt, bias=BP[0:2 * GT, _B_EPS:_B_EPS + 1])
            S16 = statp.tile([2 * GT, NTOK], BF16, tag="S16")
            with nc.allow_low_precision(reason="inv-std broadcast in bf16"):
                nc.vector.reciprocal(S16[:], SD[:])
            nc.sync.dma_start(sstg[g], S16[:])

            # ---- phase C: decoders, super-tiles of 4 ----
            Dslab = dsm.tile([2, GT * NTOK], F32, tag="Dslab")
            for i in range(GT):
                if True:
                    n = g * GT + i
                    crep, srep = creps[i], sreps[i]
                    SBC = bcp.tile([128, 2, NTOK], BF16, tag="sbc")
                    nc.sync.dma_start(SBC[:], sstg[g, 2 * i:2 * i + 2, :].unsqueeze(0).to_broadcast((128, 2, NTOK)))

                    Zcd = pz2.tile([128, NTOK], F32, tag="Z")
                    nc.tensor.matmul(Zcd[:], W[:, _C_WGCD:_C_WGCD + 128], crep[:], start=True, stop=True)
                    zc = sb1.tile([128, NTOK], F32, tag="zc")
                    nc.vector.tensor_tensor(zc[:], Zcd[:], SBC[:, 0, :], op=OP.mult)
                    gcd = sb1.tile([128, NTOK], F32R, tag="gcd")
                    nc.scalar.activation(gcd[:], zc[:], ACTF.Gelu, bias=BP[:, _B_CD1:_B_CD1 + 1])

                    Zjd = pz2.tile([128, NTOK], F32, tag="Z")
                    nc.tensor.matmul(Zjd[:], W[:, _C_JDT:_C_JDT + 128], crep[:], start=True, stop=False)
                    nc.tensor.matmul(Zjd[:], W[:, _C_JDB:_C_JDB + 128], srep[:], start=False, stop=True)
                    zj = sb1.tile([128, NTOK], F32, tag="zj")
                    nc.vector.tensor_tensor(zj[:], Zjd[:], SBC[:, 1, :], op=OP.mult)
                    gjd = sb1.tile([128, NTOK], F32R, tag="gjd")
                    nc.scalar.activation(gjd[:], zj[:], ACTF.Gelu, bias=BP[:, _B_JD1:_B_JD1 + 1])

                    LG = plg2.tile([2, NTOK], F32, tag="lg")
                    nc.tensor.matmul(LG[:], W[:, _C_DWC:_C_DWC + 2], gcd[:], start=True, stop=False)
                    nc.tensor.matmul(LG[:], W[:, _C_DWJ:_C_DWJ + 2], gjd[:], start=False, stop=True)
                    if i % 2 == 0:
                        nc.vector.tensor_copy(Dslab[:, i * NTOK:(i + 1) * NTOK], LG[:])
                    else:
                        nc.scalar.copy(Dslab[:, i * NTOK:(i + 1) * NTOK], LG[:])
            nc.sync.dma_start(d_out[g], Dslab[:])
    return nc


_CACHED = {}


def _get_nc():
    if "nc" not in _CACHED:
        nc = bacc.Bacc("TRN2", target_bir_lowering=False, debug=False)
        _build_kernel(nc)
        nc.compile()
        _CACHED["nc"] = nc
    return _CACHED["nc"]


def _host_prep(inputs):
    f32 = np.float32
    g = lambda k: np.asarray(inputs[k], f32)
    xc = g('causal_summary').reshape(B * T, F)
    xs = g('spurious_summary').reshape(B * T, F)
    labels = np.asarray(inputs['labels']).astype(np.int64)
    mask = np.asarray(inputs['valid_mask']).astype(f32)

    onehot = np.eye(2, dtype=f32)[labels]
    counts = (mask.T @ onehot).astype(f32)
    tot = counts.sum(-1)
    probs = counts / np.clip(tot, 1.0, None)[:, None]
    ent = -(probs * np.log(probs + 1e-8)).sum(-1).astype(f32)
    sv = (tot >= 2.0).astype(f32)
    lab2 = (1.0 - 2.0 * labels).astype(f32)

    cd_g, cd_b = g('cd_g'), g('cd_b')
    jd_g, jd_b = g('jd_g'), g('jd_b')
    cd_w1, jd_w1 = g('cd_w1'), g('jd_w1')
    # fold LN gain, then LN mean removal = column-centering
    Wg_cd = (cd_w1 * cd_g[:, None]).astype(f32)
    Wg_cd = (Wg_cd - Wg_cd.mean(0, keepdims=True)) * H
    bias_cd1 = (g('cd_b1') + cd_w1.T @ cd_b).astype(f32)
    Wg_jd = (jd_w1 * jd_g[:, None]).astype(f32)
    Wg_jd = (Wg_jd - Wg_jd.mean(0, keepdims=True)) * (2 * H)
    bias_jd1 = (g('jd_b1') + jd_w1.T @ jd_b).astype(f32)
    cd_w2, jd_w2 = g('cd_w2'), g('jd_w2')
    cd_b2, jd_b2 = g('cd_b2'), g('jd_b2')
    dw_cd = (cd_w2[:, 1] - cd_w2[:, 0]).astype(f32)
    dw_jd = (jd_w2[:, 1] - jd_w2[:, 0]).astype(f32)
    db_cd = f32(cd_b2[1] - cd_b2[0])
    db_jd = f32(jd_b2[1] - jd_b2[0])

    wts = np.zeros((128, WC), f32)
    wts[:, _C_CEW2:_C_CEW2 + 128] = g('ce_w2')
    wts[:, _C_SEW2:_C_SEW2 + 128] = g('se_w2')
    wts[:, _C_WGCD:_C_WGCD + 128] = Wg_cd
    wts[:, _C_JDT:_C_JDT + 128] = Wg_jd[:H]
    wts[:, _C_JDB:_C_JDB + 128] = Wg_jd[H:]
    wts[:, _C_DWC] = dw_cd
    wts[:, _C_DWJ + 1] = dw_jd
    # stat selectors: SPA = (sum_c, sum_c+sum_s), SPB = (H ssq_c, 2H (ssq_c+ssq_s))
    wts[:, _C_SEL + 0] = 1.0
    wts[:, _C_SEL + 1] = 1.0
    wts[:, _C_SEL + 3] = 1.0
    wts[:, _C_SEL + 4] = H
    wts[:, _C_SEL + 5] = 2 * H
    wts[:, _C_SEL + 7] = 2 * H

    wbf = np.zeros((128, 256), ml_dtypes.bfloat16)
    wbf[:, 0:128] = g('ce_w1').astype(ml_dtypes.bfloat16)
    wbf[:, 128:256] = g('se_w1').astype(ml_dtypes.bfloat16)

    biasp = np.zeros((128, NBP), f32)
    biasp[:, _B_CE1] = g('ce_b1')
    biasp[:, _B_CE2] = g('ce_b2')
    biasp[:, _B_SE1] = g('se_b1')
    biasp[:, _B_SE2] = g('se_b2')
    biasp[:, _B_CD1] = bias_cd1
    biasp[:, _B_JD1] = bias_jd1
    biasp[0:2 * GT:2, _B_EPS] = (H * H) * 1e-5
    biasp[1:2 * GT:2, _B_EPS] = (2 * H) * (2 * H) * 1e-5

    bf = ml_dtypes.bfloat16
    in_maps = []
    for core in range(NCORES):
        s = core * TOK
        in_maps.append({
            "xc": np.ascontiguousarray(xc[s:s + TOK]).astype(bf),
            "xs": np.ascontiguousarray(xs[s:s + TOK]).astype(bf),
            "wts": wts, "wbf": wbf, "biasp": biasp,
        })
    host = {"ent": ent, "sv": sv, "tot": tot, "mask": mask,
            "lab2": lab2, "db_cd": db_cd, "db_jd": db_jd}
    return in_maps, host


def _softplus(x):
    return np.log1p(np.exp(-np.abs(x))) + np.maximum(x, 0)


def _host_post(results, host):
    f32 = np.float32
    ent, sv, tot = host["ent"], host["sv"], host["tot"]
    mask, lab2 = host["mask"], host["lab2"]
    d_c = np.concatenate([np.asarray(r["d"], f32)[:, 0, :].reshape(TOK) for r in results])
    d_j = np.concatenate([np.asarray(r["d"], f32)[:, 1, :].reshape(TOK) for r in results])
    lab2f = np.repeat(lab2, T)
    ce_c = _softplus((d_c + host["db_cd"]) * lab2f).reshape(B, T).astype(f32)
    ce_j = _softplus((d_j + host["db_jd"]) * lab2f).reshape(B, T).astype(f32)
    reward = mask * sv[None, :] * (ent[None, :] - ce_c - LAM * np.maximum(ce_c - ce_j, 0.0))
    Sc = (mask * ce_c).sum(0)
    Sj = (mask * ce_j).sum(0)
    denom = np.clip(tot, 1.0, None)
    aux_step = 0.5 * (Sc + Sj) / denom
    n_valid = sv.sum()
    aux = (sv * aux_step).sum() / max(n_valid, 1.0) if n_valid > 0 else 0.0
    return reward.astype(f32), np.float32(aux)


def kernel(**inputs):
    nc = _get_nc()
    in_maps, host = _host_prep(inputs)
    res = run_bass_kernel_spmd(nc, in_maps, list(range(NCORES)))
    return _host_post(res.results, host)


if __name__ == "__main__":
    import reference as ref
    inputs = {k: np.asarray(v) for k, v in ref.setup_inputs().items()}
    out = kernel(**inputs)
    print(out[0].shape, out[1])


# revision 37
# speedup vs baseline: 1.1650x; 1.0075x over previous
"""Trainium2 Bass kernel for nn_DualAgentEnvironment (dual encoder/decoder CE + MI reward).

Data-parallel over B=4096 -> 8 cores x 512 rows. Each core runs a
feature-major MLP pipeline (features on partitions, tokens on the free dim),
so every layer is one PE matmul with no transposes. Host-side folds:
  - LN gain into the decoder weight; LN mean removal by column-centering it
    (the rank-1 mean correction ones (x) u equals subtracting column means)
  - LN beta + first decoder bias into one per-feature bias
  - logits head into a difference head dw = w2[:,1]-w2[:,0] (2-class CE only
    needs the logit gap: ce = softplus((l1-l0)*(1-2y)))
Device LN reduces to a per-token inv-std column scale applied after the
decoder matmul (column scaling commutes with left matmul). Per-token LN
sums are accumulated into an aligned [4,512] PSUM tile via selector-column
matmuls, batched per 8-tile group in a PSUM slab (PSUM access patterns are
exempt from the SBUF 32-partition-alignment rule, enabling shifted-row
reads), and turned into inv-std with one batched sqrt + per-tile reciprocal.
The device outputs per-token logit gaps [64,2,512]; the cheap O(B*T)
softplus/mask/entropy/reward/aux math runs on the host.

kernel(**inputs) takes FULL unsharded inputs, returns
(reward [4096,64] f32, aux_loss f32 scalar), matching reference().
"""

import numpy as np
import ml_dtypes
from contextlib import ExitStack

import concourse.bass as bass
import concourse.bacc as bacc
import concourse.mybir as mybir
import concourse.tile as tile
from concourse.bass_utils import run_bass_kernel_spmd

F32 = mybir.dt.float32
F32R = mybir.dt.float32r
BF16 = mybir.dt.bfloat16
AX = mybir.AxisListType
OP = mybir.AluOpType
ACTF = mybir.ActivationFunctionType

B, T, F, H = 4096, 64, 128, 128
LAM = 0.5
NCORES = 8
BS = B // NCORES            # 512 batch rows per core
TOK = BS * T                # 32768 tokens per core
NTOK = 512                  # tokens per tile
NT = TOK // NTOK            # 64 tiles
GT = 8                      # tiles per group (stats batching)
NG = NT // GT               # 8 groups

# weights pack column offsets (f32 [128, WC])
_C_CEW2 = 0
_C_SEW2 = 128
_C_WGCD = 256
_C_JDT = 384
_C_JDB = 512
_C_DWC = 640          # [128,2]: col0 = cd_w2[:,1]-cd_w2[:,0], col1 = 0
_C_DWJ = 642          # [128,2]: col0 = 0, col1 = jd_w2[:,1]-jd_w2[:,0]
_C_SEL = 644          # 4 selector blocks [128,2] for the LN stat sums
WC = 652

# bias pack columns (f32 [128, NBP])
(_B_CE1, _B_CE2, _B_SE1, _B_SE2, _B_CD1, _B_JD1,
 _B_KVG, _B_KVJ, _B_EPS) = range(9)
NBP = 10


def _f32r(ap):
    return ap.bitcast(F32R)


def _build_kernel(nc: bass.Bass):
    xc = nc.dram_tensor("xc", [TOK, F], BF16, kind="ExternalInput").ap()
    xs = nc.dram_tensor("xs", [TOK, F], BF16, kind="ExternalInput").ap()
    wts = nc.dram_tensor("wts", [128, WC], F32R, kind="ExternalInput").ap()
    wbf = nc.dram_tensor("wbf", [128, 256], BF16, kind="ExternalInput").ap()
    biasp = nc.dram_tensor("biasp", [128, NBP], F32, kind="ExternalInput").ap()
    d_out = nc.dram_tensor("d", [NG, 2, GT * NTOK], F32, kind="ExternalOutput").ap()
    ustg = nc.dram_tensor("ustg", [NG, 2, GT * NTOK], F32).ap()
    sstg = nc.dram_tensor("sstg", [NG, 2 * GT, NTOK], BF16).ap()

    with tile.TileContext(nc) as tc, ExitStack() as ctx:
        consts = ctx.enter_context(tc.tile_pool(name="consts", bufs=1))
        xin = ctx.enter_context(tc.tile_pool(name="xin", bufs=2))
        sb1 = ctx.enter_context(tc.tile_pool(name="sb1", bufs=2))
        rep = ctx.enter_context(tc.tile_pool(name="rep", bufs=16))
        bcp = ctx.enter_context(tc.tile_pool(name="bcp", bufs=3))
        statp = ctx.enter_context(tc.tile_pool(name="statp", bufs=2))
        dsm = ctx.enter_context(tc.tile_pool(name="dsm", bufs=2))
        pmm = ctx.enter_context(tc.tile_pool(name="pmm", bufs=4, space="PSUM"))
        psml = ctx.enter_context(tc.tile_pool(name="psml", bufs=2, space="PSUM"))
        pz2 = ctx.enter_context(tc.tile_pool(name="pz2", bufs=1, space="PSUM"))
        plg2 = ctx.enter_context(tc.tile_pool(name="plg2", bufs=1, space="PSUM"))

        # constants
        W = consts.tile([128, WC], F32R, tag="wts")
        nc.sync.dma_start(W[:], wts)
        WB = consts.tile([128, 256], BF16, tag="wbf")
        nc.sync.dma_start(WB[:], wbf)
        BP = consts.tile([128, NBP], F32, tag="biasp")
        nc.sync.dma_start(BP[:], biasp)

        sel = lambda k: W[:, _C_SEL + 2 * k:_C_SEL + 2 * (k + 1)]

        def phase_c(g, creps, sreps):
            Dslab = dsm.tile([2, GT * NTOK], F32, tag="Dslab")
            for i in range(GT):
                if True:
                    n = g * GT + i
                    crep, srep = creps[i], sreps[i]
                    SBC = bcp.tile([128, 2, NTOK], BF16, tag="sbc")
                    nc.sync.dma_start(SBC[:], sstg[g, 2 * i:2 * i + 2, :].unsqueeze(0).to_broadcast((128, 2, NTOK)))

                    Zcd = pz2.tile([128, NTOK], F32, tag="Z")
                    nc.tensor.matmul(Zcd[:], W[:, _C_WGCD:_C_WGCD + 128], crep[:], start=True, stop=True)
                    zc = sb1.tile([128, NTOK], F32, tag="zc")
                    nc.vector.tensor_tensor(zc[:], Zcd[:], SBC[:, 0, :], op=OP.mult)
                    gcd = sb1.tile([128, NTOK], F32R, tag="gcd")
                    nc.scalar.activation(gcd[:], zc[:], ACTF.Gelu, bias=BP[:, _B_CD1:_B_CD1 + 1])

                    Zjd = pz2.tile([128, NTOK], F32, tag="Z")
                    nc.tensor.matmul(Zjd[:], W[:, _C_JDT:_C_JDT + 128], crep[:], start=True, stop=False)
                    nc.tensor.matmul(Zjd[:], W[:, _C_JDB:_C_JDB + 128], srep[:], start=False, stop=True)
                    zj = sb1.tile([128, NTOK], F32, tag="zj")
                    nc.vector.tensor_tensor(zj[:], Zjd[:], SBC[:, 1, :], op=OP.mult)
                    gjd = sb1.tile([128, NTOK], F32R, tag="gjd")
                    nc.scalar.activation(gjd[:], zj[:], ACTF.Gelu, bias=BP[:, _B_JD1:_B_JD1 + 1])

                    LG = plg2.tile([2, NTOK], F32, tag="lg")
                    nc.tensor.matmul(LG[:], W[:, _C_DWC:_C_DWC + 2], gcd[:], start=True, stop=False)
                    nc.tensor.matmul(LG[:], W[:, _C_DWJ:_C_DWJ + 2], gjd[:], start=False, stop=True)
                    if i % 2 == 0:
                        nc.vector.tensor_copy(Dslab[:, i * NTOK:(i + 1) * NTOK], LG[:])
                    else:
                        nc.scalar.copy(Dslab[:, i * NTOK:(i + 1) * NTOK], LG[:])
            nc.sync.dma_start(d_out[g], Dslab[:])

        pending = None
        for g in range(NG):
            creps, sreps = [], []
            Uslab = dsm.tile([2, GT * NTOK], F32, tag="Uslab")
            # ---- phase A: encoders + LN stat sums, super-tiles of 4 ----
            for st in range(GT // 2):
                base = g * GT + st * 2
                xc4 = xin.tile([128, 2 * NTOK], BF16, tag="xc4")
                nc.sync.dma_start_transpose(xc4[:], xc[base * NTOK:(base + 2) * NTOK, :])
                xs4 = xin.tile([128, 2 * NTOK], BF16, tag="xs4")
                nc.sync.dma_start_transpose(xs4[:], xs[base * NTOK:(base + 2) * NTOK, :])
                for j in range(2):
                    xcT = xc4[:, j * NTOK:(j + 1) * NTOK]
                    xsT = xs4[:, j * NTOK:(j + 1) * NTOK]
                    c1 = pmm.tile([128, NTOK], F32, tag="mm")
                    nc.tensor.matmul(c1[:], WB[:, 0:128], xcT, start=True, stop=True)
                    g1c = sb1.tile([128, NTOK], F32R, tag="g1c")
                    nc.scalar.activation(g1c[:], c1[:], ACTF.Gelu, bias=BP[:, _B_CE1:_B_CE1 + 1])
                    c2 = pmm.tile([128, NTOK], F32, tag="mm")
                    nc.tensor.matmul(c2[:], W[:, _C_CEW2:_C_CEW2 + 128], g1c[:], start=True, stop=True)
                    crep = rep.tile([128, NTOK], F32R, tag="crep")
                    nc.vector.tensor_scalar(crep[:], c2[:], BP[:, _B_CE2:_B_CE2 + 1], None, op0=OP.add)
                    sqc = sb1.tile([128, NTOK], F32R, tag="sqc")
                    nc.gpsimd.tensor_tensor(sqc[:], crep[:], crep[:], op=OP.mult)

                    s1 = pmm.tile([128, NTOK], F32, tag="mm")
                    nc.tensor.matmul(s1[:], WB[:, 128:256], xsT, start=True, stop=True)
                    g1s = sb1.tile([128, NTOK], F32R, tag="g1s")
                    nc.scalar.activation(g1s[:], s1[:], ACTF.Gelu, bias=BP[:, _B_SE1:_B_SE1 + 1])
                    s2 = pmm.tile([128, NTOK], F32, tag="mm")
                    nc.tensor.matmul(s2[:], W[:, _C_SEW2:_C_SEW2 + 128], g1s[:], start=True, stop=True)
                    srep = rep.tile([128, NTOK], F32R, tag="srep")
                    nc.vector.tensor_scalar(srep[:], s2[:], BP[:, _B_SE2:_B_SE2 + 1], None, op0=OP.add)
                    sqs = sb1.tile([128, NTOK], F32R, tag="sqs")
                    nc.gpsimd.tensor_tensor(sqs[:], srep[:], srep[:], op=OP.mult)

                    # LN stats: SPA = (sum_c, sum_j), SPB = (H ssq_c, 2H ssq_j)
                    SPA = psml.tile([2, NTOK], F32, tag="sml")
                    nc.tensor.matmul(SPA[:], sel(0), crep[:], start=True, stop=False)
                    nc.tensor.matmul(SPA[:], sel(1), srep[:], start=False, stop=True)
                    SPB = psml.tile([2, NTOK], F32, tag="sml")
                    nc.tensor.matmul(SPB[:], sel(2), sqc[:], start=True, stop=False)
                    nc.tensor.matmul(SPB[:], sel(3), sqs[:], start=False, stop=True)
                    # u = n*ssq - sum^2, packed into the group slab (free dim)
                    T1 = sb1.tile([2, NTOK], F32, tag="T1")
                    nc.scalar.activation(T1[:], SPA[:], ACTF.Square)
                    i = st * 2 + j
                    nc.vector.tensor_tensor(Uslab[:, i * NTOK:(i + 1) * NTOK], SPB[:], T1[:], op=OP.subtract)
                    creps.append(crep)
                    sreps.append(srep)

            # batched inv-std for the group via the DRAM staging round-trip
            nc.sync.dma_start(ustg[g], Uslab[:])
            USL = statp.tile([2 * GT, NTOK], F32, tag="USL")
            nc.sync.dma_start(USL[:], ustg[g].rearrange("r (i c) -> i r c", c=NTOK))
            SD = statp.tile([2 * GT, NTOK], F32, tag="SD")
            nc.scalar.activation(SD[:], USL[:], ACTF.Sqrt, bias=BP[0:2 * GT, _B_EPS:_B_EPS + 1])
            S16 = statp.tile([2 * GT, NTOK], BF16, tag="S16")
            with nc.allow_low_precision(reason="inv-std broadcast in bf16"):
                nc.vector.reciprocal(S16[:], SD[:])
            nc.sync.dma_start(sstg[g], S16[:])

            if pending is not None:
                phase_c(*pending)
            pending = (g, creps, sreps)
        phase_c(*pending)
    return nc


_CACHED = {}


def _get_nc():
    if "nc" not in _CACHED:
        nc = bacc.Bacc("TRN2", target_bir_lowering=False, debug=False)
        _build_kernel(nc)
        nc.compile()
        _CACHED["nc"] = nc
    return _CACHED["nc"]


def _host_prep(inputs):
    f32 = np.float32
    g = lambda k: np.asarray(inputs[k], f32)
    xc = g('causal_summary').reshape(B * T, F)
    xs = g('spurious_summary').reshape(B * T, F)
    labels = np.asarray(inputs['labels']).astype(np.int64)
    mask = np.asarray(inputs['valid_mask']).astype(f32)

    onehot = np.eye(2, dtype=f32)[labels]
    counts = (mask.T @ onehot).astype(f32)
    tot = counts.sum(-1)
    probs = counts / np.clip(tot, 1.0, None)[:, None]
    ent = -(probs * np.log(probs + 1e-8)).sum(-1).astype(f32)
    sv = (tot >= 2.0).astype(f32)
    lab2 = (1.0 - 2.0 * labels).astype(f32)

    cd_g, cd_b = g('cd_g'), g('cd_b')
    jd_g, jd_b = g('jd_g'), g('jd_b')
    cd_w1, jd_w1 = g('cd_w1'), g('jd_w1')
    # fold LN gain, then LN mean removal = column-centering
    Wg_cd = (cd_w1 * cd_g[:, None]).astype(f32)
    Wg_cd = (Wg_cd - Wg_cd.mean(0, keepdims=True)) * H
    bias_cd1 = (g('cd_b1') + cd_w1.T @ cd_b).astype(f32)
    Wg_jd = (jd_w1 * jd_g[:, None]).astype(f32)
    Wg_jd = (Wg_jd - Wg_jd.mean(0, keepdims=True)) * (2 * H)
    bias_jd1 = (g('jd_b1') + jd_w1.T @ jd_b).astype(f32)
    cd_w2, jd_w2 = g('cd_w2'), g('jd_w2')
    cd_b2, jd_b2 = g('cd_b2'), g('jd_b2')
    dw_cd = (cd_w2[:, 1] - cd_w2[:, 0]).astype(f32)
    dw_jd = (jd_w2[:, 1] - jd_w2[:, 0]).astype(f32)
    db_cd = f32(cd_b2[1] - cd_b2[0])
    db_jd = f32(jd_b2[1] - jd_b2[0])

    wts = np.zeros((128, WC), f32)
    wts[:, _C_CEW2:_C_CEW2 + 128] = g('ce_w2')
    wts[:, _C_SEW2:_C_SEW2 + 128] = g('se_w2')
    wts[:, _C_WGCD:_C_WGCD + 128] = Wg_cd
    wts[:, _C_JDT:_C_JDT + 128] = Wg_jd[:H]
    wts[:, _C_JDB:_C_JDB + 128] = Wg_jd[H:]
    wts[:, _C_DWC] = dw_cd
    wts[:, _C_DWJ + 1] = dw_jd
    # stat selectors: SPA = (sum_c, sum_c+sum_s), SPB = (H ssq_c, 2H (ssq_c+ssq_s))
    wts[:, _C_SEL + 0] = 1.0
    wts[:, _C_SEL + 1] = 1.0
    wts[:, _C_SEL + 3] = 1.0
    wts[:, _C_SEL + 4] = H
    wts[:, _C_SEL + 5] = 2 * H
    wts[:, _C_SEL + 7] = 2 * H

    wbf = np.zeros((128, 256), ml_dtypes.bfloat16)
    wbf[:, 0:128] = g('ce_w1').astype(ml_dtypes.bfloat16)
    wbf[:, 128:256] = g('se_w1').astype(ml_dtypes.bfloat16)

    biasp = np.zeros((128, NBP), f32)
    biasp[:, _B_CE1] = g('ce_b1')
    biasp[:, _B_CE2] = g('ce_b2')
    biasp[:, _B_SE1] = g('se_b1')
    biasp[:, _B_SE2] = g('se_b2')
    biasp[:, _B_CD1] = bias_cd1
    biasp[:, _B_JD1] = bias_jd1
    biasp[0:2 * GT:2, _B_EPS] = (H * H) * 1e-5
    biasp[1:2 * GT:2, _B_EPS] = (2 * H) * (2 * H) * 1e-5

    bf = ml_dtypes.bfloat16
    in_maps = []
    for core in range(NCORES):
        s = core * TOK
        in_maps.append({
            "xc": np.ascontiguousarray(xc[s:s + TOK]).astype(bf),
            "xs": np.ascontiguousarray(xs[s:s + TOK]).astype(bf),
            "wts": wts, "wbf": wbf, "biasp": biasp,
        })
    host = {"ent": ent, "sv": sv, "tot": tot, "mask": mask,
            "lab2": lab2, "db_cd": db_cd, "db_jd": db_jd}
    return in_maps, host


def _softplus(x):
    return np.log1p(np.exp(-np.abs(x))) + np.maximum(x, 0)


def _host_post(results, host):
    f32 = np.float32
    ent, sv, tot = host["ent"], host["sv"], host["tot"]
    mask, lab2 = host["mask"], host["lab2"]
    d_c = np.concatenate([np.asarray(r["d"], f32)[:, 0, :].reshape(TOK) for r in results])
    d_j = np.concatenate([np.asarray(r["d"], f32)[:, 1, :].reshape(TOK) for r in results])
    lab2f = np.repeat(lab2, T)
    ce_c = _softplus((d_c + host["db_cd"]) * lab2f).reshape(B, T).astype(f32)
    ce_j = _softplus((d_j + host["db_jd"]) * lab2f).reshape(B, T).astype(f32)
    reward = mask * sv[None, :] * (ent[None, :] - ce_c - LAM * np.maximum(ce_c - ce_j, 0.0))
    Sc = (mask * ce_c).sum(0)
    Sj = (mask * ce_j).sum(0)
    denom = np.clip(tot, 1.0, None)
    aux_step = 0.5 * (Sc + Sj) / denom
    n_valid = sv.sum()
    aux = (sv * aux_step).sum() / max(n_valid, 1.0) if n_valid > 0 else 0.0
    return reward.astype(f32), np.float32(aux)


def kernel(**inputs):
    nc = _get_nc()
    in_maps, host = _host_prep(inputs)
    res = run_bass_kernel_spmd(nc, in_maps, list(range(NCORES)))
    return _host_post(res.results, host)


if __name__ == "__main__":
    import reference as ref
    inputs = {k: np.asarray(v) for k, v in ref.setup_inputs().items()}
    out = kernel(**inputs)
    print(out[0].shape, out[1])
